# revision 26
# baseline (speedup 1.0000x reference)
"""Self-contained CLAGCN kernel for 8 Trainium2 NeuronCores (v2).

kernel(**inputs) takes the full (unsharded) numpy inputs and returns the
full outputs (out, gnn_prop1, gnn_prop2), matching reference().

Node rows are partitioned contiguously across the 8 cores (graph
parallel); edges live on the core that owns their destination row.
First-layer supports (x @ W1, both branches) are computed per-shard as
bf16 and all-gathered per view; each core runs its spmm as dma_gather
(source-row fetch) + one-hot matmul segment-sum into PSUM windows of 128
destinations.  One-hot "S" blocks are generated on-device by the vector
engine from per-slot (dest, val) pairs (iota compare), instead of being
streamed from HBM.  The second layer repeats the pattern on the gathered
[h1W2 | h2W2] table.  Tiny sigmoid gates are computed on the core owning
the last node and broadcast with small all-gathers; scalar broadcasts go
through a 1xK matmul so the gpsimd engine stays dedicated to gather
descriptor generation (the critical path).
"""

import sys

sys.path.insert(0, "/opt/trn_rl_repo")
import numpy as np
import ml_dtypes

import concourse.bass as bass
import concourse.mybir as mybir
import concourse.tile as tile
from concourse import bacc
from concourse import bass_utils

BF16 = mybir.dt.bfloat16
F32 = mybir.dt.float32
I16 = mybir.dt.int16
AOT = mybir.AluOpType
ACT = mybir.ActivationFunctionType
P = 128
EPS = 1e-12
PIN_SLOT = 96  # partition slot of the last local node


class Cfg:
    def __init__(self, N=50000, NFEAT=256, NHID=128, NCLASS=16, NCORES=8,
                 TH_CORES=5, BPW_LO=11, BPW_HI=7, L1B=18, L2B=16):
        self.N, self.NFEAT, self.NHID, self.NCLASS, self.NCORES = N, NFEAT, NHID, NCLASS, NCORES
        assert N % NCORES == 0
        self.Nc = N // NCORES                 # nodes per core
        self.NT = (self.Nc + P - 1) // P      # dest windows per core
        self.Ncp = self.NT * P                # padded local nodes
        self.TH_CORES = TH_CORES              # lo/hi table split at this core boundary
        self.TH = TH_CORES * self.Ncp         # row threshold in the gathered tables
        self.BPW = (BPW_LO, BPW_HI)           # edge blocks per window per stream
        self.NBS = (self.NT * BPW_LO, self.NT * BPW_HI)
        self.L1B = L1B                        # L1 gather batch (blocks per dma_gather)
        self.L2B = L2B
        self.NH2 = 2 * NHID                   # 256: concat branch a/b (per-view table row)
        self.QW = 128                         # q table row elems (bf16; 2*NCLASS used)
        self.Nq = NCORES * self.Ncp
        assert self.TH < 32768 and self.Nq - self.TH < 32768


DEFAULT_CFG = Cfg()


# ---------------------------------------------------------------- host prep

def _pack_core(r1, r2, cfg):
    """Assign local dest nodes to (window, slot), balancing joint degree.
    Returns perm[Nc] -> permuted index (window*128 + slot)."""
    import heapq
    Nc, NT = cfg.Nc, cfg.NT
    deg1 = np.bincount(r1, minlength=Nc)
    deg2 = np.bincount(r2, minlength=Nc)
    degj = deg1 + deg2
    last = Nc - 1
    order = np.argsort(-degj, kind="stable")
    order = order[order != last]

    win_members = [[] for _ in range(NT)]
    win_members[NT - 1].append(last)
    heap = [(int(degj[last]) if w == NT - 1 else 0, w) for w in range(NT)]
    heapq.heapify(heap)
    for d in order:
        while True:
            load, w = heapq.heappop(heap)
            if len(win_members[w]) < P:
                break
        win_members[w].append(int(d))
        heapq.heappush(heap, (load + int(degj[d]), w))

    perm = np.empty(Nc, np.int64)
    for w in range(NT):
        slots = [s for s in range(P) if not (w == NT - 1 and s == PIN_SLOT)]
        members = win_members[w]
        if w == NT - 1:
            perm[last] = w * P + PIN_SLOT
            members = [d for d in members if d != last]
        assert len(members) <= len(slots)
        for s, d in zip(slots, members):
            perm[d] = w * P + s
    return perm


def _stream_arrays(r, c, v, perm, bpw, cfg, pad_col=0):
    """Lay out one stream's edges into slot arrays (window blocks in WORDER
    processing order so gather calls are consumed sequentially).
    Returns (col_global [NBs*128], destval [128, NBs*2] f32)."""
    NT = cfg.NT
    NBs = NT * bpw
    worder = [NT - 1] + list(range(NT - 1))
    worder_pos = np.empty(NT, np.int64)
    for i, w in enumerate(worder):
        worder_pos[w] = i
    pidx = perm[r]
    w_e = worder_pos[pidx // P]
    order = np.argsort(w_e, kind="stable")
    w_s = w_e[order]
    counts = np.bincount(w_s, minlength=NT)
    assert counts.max() <= bpw * P, f"stream window overflow: {counts.max()} > {bpw * P}"
    starts = np.concatenate([[0], np.cumsum(counts)[:-1]])
    within = np.arange(len(order)) - starts[w_s]
    slot = w_s * (bpw * P) + within

    col_flat = np.full(NBs * P, pad_col, np.int64)
    dest_flat = np.zeros(NBs * P, np.float32)
    val_flat = np.zeros(NBs * P, np.float32)
    col_flat[slot] = c[order]
    dest_flat[slot] = (pidx[order] % P).astype(np.float32)
    val_flat[slot] = v[order]
    return (col_flat,
            dest_flat.reshape(NBs, P).T.copy(),
            val_flat.reshape(NBs, P).T.copy())


def _wrap16(rows):
    """int16 dma_gather index layout: [128, n/16], position i -> [i%16, i//16]."""
    assert rows.min() >= 0 and rows.max() < 32768, (rows.min(), rows.max())
    return np.tile(rows.astype(np.int16).reshape(-1, 16).T, (8, 1)).copy()


def prepare(inputs, cfg):
    """Split + pack everything.  Returns (in_maps, perms)."""
    bf = ml_dtypes.bfloat16
    N, Nc, Ncp, NFEAT = cfg.N, cfg.Nc, cfg.Ncp, cfg.NFEAT
    NC = cfg.NCORES

    row1 = np.asarray(inputs["row1"]); col1 = np.asarray(inputs["col1"]); ew1 = np.asarray(inputs["ew1"])
    row2 = np.asarray(inputs["row2"]); col2 = np.asarray(inputs["col2"]); ew2 = np.asarray(inputs["ew2"])

    E1 = [np.nonzero(row1 // Nc == cc)[0] for cc in range(NC)]
    E2 = [np.nonzero(row2 // Nc == cc)[0] for cc in range(NC)]

    perms = []
    for cc in range(NC):
        r1 = row1[E1[cc]] - cc * Nc
        r2 = row2[E2[cc]] - cc * Nc
        perms.append(_pack_core(r1, r2, cfg))

    W1a = np.asarray(inputs["W1a"]).astype(bf)
    W1b = np.asarray(inputs["W1b"]).astype(bf)
    Wq = np.asarray(inputs["W2"]).astype(bf)
    bias_row = np.concatenate([np.asarray(inputs["b1a"]), np.asarray(inputs["b1b"])])[None, :].astype(np.float32)
    b2bc = np.broadcast_to(np.asarray(inputs["b2"])[None, :], (P, cfg.NCLASS)).astype(np.float32).copy()
    fw1 = np.asarray(inputs["fw1_w"])[:, 0][None, :].astype(np.float32).copy()
    fw2 = np.asarray(inputs["fw2_w"])[:, 0][None, :].astype(np.float32).copy()
    g1w = np.asarray(inputs["g1_w"]).T.astype(np.float32).copy()
    g2w = np.asarray(inputs["g2_w"]).T.astype(np.float32).copy()
    gb = np.array([[float(inputs["fw1_b"][0]), float(inputs["fw2_b"][0]),
                    float(inputs["g1_b"][0]), float(inputs["g2_b"][0])]], np.float32)
    ones_row = np.ones((1, P), np.float32)
    ident = np.eye(P, dtype=np.float32).astype(bf)

    def xT(name, cc):
        xs = np.asarray(inputs[name])[cc * Nc:(cc + 1) * Nc].astype(bf)
        out = np.zeros((NFEAT, Ncp), bf)
        out[:, :Nc] = xs.T
        return out

    in_maps = []
    for cc in range(NC):
        perm = perms[cc]
        im = {
            "xT1a": xT("x1a", cc), "xT1b": xT("x1b", cc),
            "xT2a": xT("x2a", cc), "xT2b": xT("x2b", cc),
            "W1a": W1a, "W1b": W1b, "Wq": Wq,
            "bias_row": bias_row, "b2bc": b2bc,
            "fw1": fw1, "fw2": fw2, "g1w": g1w, "g2w": g2w, "gb": gb,
            "ones_row": ones_row, "ident": ident,
        }
        for a, (Ea, rowa, cola, ewa) in enumerate(((E1[cc], row1, col1, ew1),
                                                   (E2[cc], row2, col2, ew2))):
            r = rowa[Ea] - cc * Nc
            c = cola[Ea]
            v = ewa[Ea]
            lo = (c // Nc) < cfg.TH_CORES
            for s, mask in enumerate((lo, ~lo)):
                colg, dest, val = _stream_arrays(r[mask], c[mask], v[mask],
                                                 perm, cfg.BPW[s], cfg,
                                                 pad_col=s * cfg.TH_CORES * Nc)
                nbs = cfg.NBS[s]
                S = np.zeros((P, nbs * P), ml_dtypes.bfloat16)
                pidx_r = np.arange(P)[:, None]
                colidx = np.arange(nbs)[None, :] * P + dest.astype(np.int64)
                S[pidx_r, colidx] = val.astype(ml_dtypes.bfloat16)
                im[f"S{a}{s}"] = S
                oc = colg // Nc
                loc = colg % Nc
                srow = oc * Ncp + loc
                qrow = np.empty_like(colg)
                for o in range(NC):
                    m = oc == o
                    qrow[m] = o * Ncp + perms[o][loc[m]]
                off = s * cfg.TH
                im[f"ix1_{a}{s}"] = _wrap16(srow - off)
                im[f"ix2_{a}{s}"] = _wrap16(qrow - off)
        in_maps.append(im)
    return in_maps, perms


# ---------------------------------------------------------------- program

def build(cfg, use_bias=False):
    NFEAT, NHID, NCLASS = cfg.NFEAT, cfg.NHID, cfg.NCLASS
    NT, Ncp, TH, Nq = cfg.NT, cfg.Ncp, cfg.TH, cfg.Nq
    NH2, QW = cfg.NH2, cfg.QW
    NC = cfg.NCORES
    RG = [list(range(NC))]
    KH = NFEAT // P
    assert KH == 2 and NHID == P
    WORDER = [NT - 1] + list(range(NT - 1))   # pin window first (gates early)
    NQUEUE = 4

    nc = bacc.Bacc("TRN2", target_bir_lowering=False, debug=False, num_devices=NC,
                   num_swdge_queues=4)

    t_x = {n: nc.dram_tensor(n, [NFEAT, Ncp], BF16, kind="ExternalInput")
           for n in ("xT1a", "xT1b", "xT2a", "xT2b")}
    t_W1a = nc.dram_tensor("W1a", [NFEAT, NHID], BF16, kind="ExternalInput")
    t_W1b = nc.dram_tensor("W1b", [NFEAT, NHID], BF16, kind="ExternalInput")
    t_Wq = nc.dram_tensor("Wq", [NH2, NCLASS], BF16, kind="ExternalInput")
    t_bias_row = nc.dram_tensor("bias_row", [1, NH2], F32, kind="ExternalInput")
    t_b2bc = nc.dram_tensor("b2bc", [P, NCLASS], F32, kind="ExternalInput")
    t_fw1 = nc.dram_tensor("fw1", [1, NH2], F32, kind="ExternalInput")
    t_fw2 = nc.dram_tensor("fw2", [1, NH2], F32, kind="ExternalInput")
    t_g1w = nc.dram_tensor("g1w", [1, NCLASS], F32, kind="ExternalInput")
    t_g2w = nc.dram_tensor("g2w", [1, NCLASS], F32, kind="ExternalInput")
    t_gb = nc.dram_tensor("gb", [1, 4], F32, kind="ExternalInput")
    t_ones = nc.dram_tensor("ones_row", [1, P], F32, kind="ExternalInput")
    t_ident = nc.dram_tensor("ident", [P, P], BF16, kind="ExternalInput")
    t_dest, t_ix1, t_ix2 = {}, {}, {}
    for a in range(2):
        for s in range(2):
            nbs = cfg.NBS[s]
            t_dest[a, s] = nc.dram_tensor(f"S{a}{s}", [P, nbs * P], BF16, kind="ExternalInput")
            t_ix1[a, s] = nc.dram_tensor(f"ix1_{a}{s}", [P, nbs * 8], I16, kind="ExternalInput")
            t_ix2[a, s] = nc.dram_tensor(f"ix2_{a}{s}", [P, nbs * 8], I16, kind="ExternalInput")
    t_out = {n: nc.dram_tensor(n, [P, NT * NCLASS], F32, kind="ExternalOutput")
             for n in ("o_out", "o_g1", "o_g2")}

    streams = (0, 1)
    qcount = [0]

    def plan_calls(B):
        """Merged (stream, call_idx, nblk) order advancing window coverage evenly."""
        ncalls = {s: (cfg.NBS[s] + B - 1) // B for s in streams}
        nxt = {s: 0 for s in streams}
        order = []
        while any(nxt[s] < ncalls[s] for s in streams):
            cand = [s for s in streams if nxt[s] < ncalls[s]]
            s = min(cand, key=lambda s: nxt[s] * B / cfg.BPW[s])
            j = nxt[s]
            order.append((s, j, min(B, cfg.NBS[s] - j * B)))
            nxt[s] += 1
        return order

    def issue_call(gp, tag, ix_sb, table_ap, elem, step, B, s, j, nblk, bufs):
        xg = gp.tile([P, B, elem], BF16, tag=f"{tag}{s}", name=f"{tag}{s}", bufs=bufs)
        nc.gpsimd.dma_gather(
            out_ap=xg[:, 0:nblk, :],
            in_ap=table_ap,
            idxs_ap=ix_sb[:, j * B * 8: (j * B + nblk) * 8],
            num_idxs=nblk * P, num_idxs_reg=nblk * P,
            elem_size=elem, elem_step=step, single_packet=False,
            queue_num=qcount[0] % NQUEUE)
        qcount[0] += 1
        return xg

    with tile.TileContext(nc) as tc:
        with tc.tile_pool(name="dram", bufs=1, space="DRAM") as dram, \
             tc.tile_pool(name="const", bufs=1) as cp:

            s_local = [dram.tile([Ncp, NH2], BF16, name=f"s_local{v}") for v in range(2)]
            s_tab = [dram.tile([Nq, NH2], BF16, addr_space="Shared", name=f"s_tab{v}") for v in range(2)]
            q_local = dram.tile([Ncp, QW], BF16)
            q_tab = dram.tile([Nq, QW], BF16, addr_space="Shared")
            wg_local = dram.tile([1, 8], F32)
            wg_all = dram.tile([NC, 8], F32, addr_space="Shared")
            ag_local = dram.tile([1, 8], F32)
            ag_all = dram.tile([NC, 8], F32, addr_space="Shared")

            w1a_sb = [cp.tile([P, NHID], BF16, name=f"w1a{kh}") for kh in range(KH)]
            w1b_sb = [cp.tile([P, NHID], BF16, name=f"w1b{kh}") for kh in range(KH)]
            wq_sb = [cp.tile([P, NCLASS], BF16, name=f"wq{kh}") for kh in range(KH)]
            for kh in range(KH):
                nc.sync.dma_start(w1a_sb[kh][:], t_W1a.ap()[kh * P:(kh + 1) * P, :])
                nc.sync.dma_start(w1b_sb[kh][:], t_W1b.ap()[kh * P:(kh + 1) * P, :])
                nc.sync.dma_start(wq_sb[kh][:], t_Wq.ap()[kh * P:(kh + 1) * P, :])
            bias_row = cp.tile([1, NH2], F32); nc.sync.dma_start(bias_row[:], t_bias_row.ap())
            b2bc = cp.tile([P, NCLASS], F32); nc.sync.dma_start(b2bc[:], t_b2bc.ap())
            fw1 = cp.tile([1, NH2], F32); nc.sync.dma_start(fw1[:], t_fw1.ap())
            fw2 = cp.tile([1, NH2], F32); nc.sync.dma_start(fw2[:], t_fw2.ap())
            g1w = cp.tile([1, NCLASS], F32); nc.sync.dma_start(g1w[:], t_g1w.ap())
            g2w = cp.tile([1, NCLASS], F32); nc.sync.dma_start(g2w[:], t_g2w.ap())
            gb = cp.tile([1, 4], F32); nc.sync.dma_start(gb[:], t_gb.ap())
            ones_row = cp.tile([1, P], F32); nc.sync.dma_start(ones_row[:], t_ones.ap())
            ident = cp.tile([P, P], BF16); nc.sync.dma_start(ident[:], t_ident.ap())
            wgbc = cp.tile([P, 2], F32)
            agbc = cp.tile([P, 2], F32)

            ix1_sb, ix2_sb = {}, {}
            for a in range(2):
                for s in streams:
                    nbs = cfg.NBS[s]
                    ix1_sb[a, s] = cp.tile([P, nbs * 8], I16, name=f"ix1sb{a}{s}")
                    ix2_sb[a, s] = cp.tile([P, nbs * 8], I16, name=f"ix2sb{a}{s}")

            # q accumulator (both views); table cols 2*NCLASS..QW are never read
            q_sb = cp.tile([P, NT, 2 * NCLASS], BF16)

            # ---- phase 1: dense supports s_v = [x_va @ W1a | x_vb @ W1b]
            with tc.tile_pool(name="dense", bufs=1) as dp, \
                 tc.tile_pool(name="dense_ps", bufs=2, space="PSUM") as dps:
                for v in range(2):
                    xa = [dp.tile([P, Ncp], BF16, tag=f"xa{kh}", name=f"xa{kh}") for kh in range(KH)]
                    xb = [dp.tile([P, Ncp], BF16, tag=f"xb{kh}", name=f"xb{kh}") for kh in range(KH)]
                    for kh in range(KH):
                        nc.sync.dma_start(xa[kh][:], t_x[f"xT{v+1}a"].ap()[kh * P:(kh + 1) * P, :])
                        nc.sync.dma_start(xb[kh][:], t_x[f"xT{v+1}b"].ap()[kh * P:(kh + 1) * P, :])
                    for t in range(NT):
                        ts = bass.ts(t, P)
                        ps = dps.tile([P, NH2], F32, tag="dps")
                        for kh in range(KH):
                            nc.tensor.matmul(ps[:, 0:NHID], lhsT=xa[kh][:, ts], rhs=w1a_sb[kh][:],
                                             start=(kh == 0), stop=(kh == KH - 1))
                        for kh in range(KH):
                            nc.tensor.matmul(ps[:, NHID:NH2], lhsT=xb[kh][:, ts], rhs=w1b_sb[kh][:],
                                             start=(kh == 0), stop=(kh == KH - 1))
                        s_sb = dp.tile([P, NH2], BF16, tag="s_sb", bufs=3)
                        nc.vector.tensor_copy(s_sb[:], ps[:])
                        nc.sync.dma_start(s_local[v][t * P:(t + 1) * P, :], s_sb[:])
                    # per-view all-gather as soon as this view's supports are done
                    nc.gpsimd.collective_compute(
                        "AllGather", AOT.bypass, replica_groups=RG,
                        ins=[s_local[v].opt()], outs=[s_tab[v].opt()])
                    if v == 0:
                        for a in range(2):
                            for s in streams:
                                nc.sync.dma_start(ix1_sb[a, s][:], t_ix1[a, s].ap())
                        for a in range(2):
                            for s in streams:
                                nc.sync.dma_start(ix2_sb[a, s][:], t_ix2[a, s].ap())

            # ---- phase 2: L1 spmm -> hT (transposed, bf16, SBUF-resident), fused q
            with tc.tile_pool(name="hpool", bufs=1) as hp:
                h_nm = [hp.tile([P, NT, NH2], BF16, name=f"hnm{a}") for a in range(2)]

                with tc.tile_pool(name="l1g", bufs=1) as gp, \
                     tc.tile_pool(name="l1sc", bufs=1) as scp, \
                     tc.tile_pool(name="l1ps", bufs=2, space="PSUM") as l1ps, \
                     tc.tile_pool(name="qps_pool", bufs=2, space="PSUM") as qps, \
                     tc.tile_pool(name="gate", bufs=1) as gt, \
                     tc.tile_pool(name="gate_ps", bufs=1, space="PSUM") as gps:
                    for a in range(2):
                        xg_tiles = {s: {} for s in streams}
                        call_plan = plan_calls(cfg.L1B)

                        def l1_window(wi, w, a=a, xg_tiles=xg_tiles):
                            ps = l1ps.tile([P, NH2], F32, tag="l1ps", bufs=2)
                            if use_bias:
                                nc.tensor.matmul(ps[:], lhsT=ones_row[:], rhs=bias_row[:],
                                                 start=True, stop=False)
                            nblk_tot = sum(cfg.BPW[s] for s in streams)
                            ki = 0
                            for s in streams:
                                bpw = cfg.BPW[s]
                                Sc = scp.tile([P, bpw * P], BF16, tag=f"sc{s}", name=f"sc{s}", bufs=4)
                                eng = nc.scalar if s == 0 else nc.sync
                                eng.dma_start(Sc[:], t_dest[a, s].ap()[:, wi * bpw * P:(wi + 1) * bpw * P])
                                for k in range(bpw):
                                    b = wi * bpw + k
                                    xg = xg_tiles[s][b // cfg.L1B]
                                    sl = b % cfg.L1B
                                    nc.tensor.matmul(ps[:], lhsT=Sc[:, k * P:(k + 1) * P],
                                                     rhs=xg[:, sl, :],
                                                     start=(not use_bias and ki == 0),
                                                     stop=(ki == nblk_tot - 1))
                                    ki += 1
                            nc.vector.tensor_scalar(out=h_nm[a][:, w, :], in0=ps[:],
                                                    scalar1=0.0, scalar2=None, op0=AOT.max)
                            # q: transpose h window (PE) then two q matmuls
                            trs = []
                            for kh in range(KH):
                                pst = qps.tile([P, P], F32, tag="pst")
                                nc.tensor.matmul(pst[:], lhsT=h_nm[a][:, w, kh * P:(kh + 1) * P],
                                                 rhs=ident[:], start=True, stop=True)
                                tr = scp.tile([P, P], BF16, tag="tr", bufs=3, name="tr")
                                nc.vector.tensor_copy(tr[:], pst[:])
                                trs.append(tr)
                            qp = qps.tile([P, NCLASS], F32, tag="qps")
                            for kh in range(KH):
                                nc.tensor.matmul(qp[:], lhsT=trs[kh][:], rhs=wq_sb[kh][:],
                                                 start=(kh == 0), stop=(kh == KH - 1))
                            nc.vector.tensor_copy(q_sb[:, w, a * NCLASS:(a + 1) * NCLASS], qp[:])
                            if a == 1 and w == NT - 1:
                                # gate-1 SB computation (collective issued in the call stream)
                                lab = gt.tile([1, 2], F32)
                                junkg = gt.tile([1, NH2], F32, name="junkg")
                                dotg = gt.tile([1, 2], F32, name="dotg")
                                h_last = [gt.tile([1, NH2], BF16, name=f"h_last{aa}") for aa in range(2)]
                                for aa in range(2):
                                    nc.sync.dma_start(h_last[aa][:], h_nm[aa][PIN_SLOT:PIN_SLOT + 1, NT - 1, :])
                                for aa, fw in ((0, fw1), (1, fw2)):
                                    nc.vector.tensor_tensor(out=junkg[:], in0=h_last[aa][:],
                                                            in1=fw[:], op=AOT.mult)
                                    nc.vector.tensor_reduce(out=dotg[0:1, aa:aa + 1], in_=junkg[:],
                                                            op=AOT.add, axis=mybir.AxisListType.X)
                                    nc.scalar.activation(lab[0:1, aa:aa + 1], dotg[0:1, aa:aa + 1],
                                                         ACT.Sigmoid, bias=gb[0:1, aa:aa + 1], scale=1.0)
                                ssum = gt.tile([1, 1], F32)
                                nc.vector.tensor_tensor(out=ssum[:], in0=lab[0:1, 0:1], in1=lab[0:1, 1:2], op=AOT.add)
                                nc.vector.tensor_scalar(out=ssum[:], in0=ssum[:], scalar1=EPS, scalar2=None, op0=AOT.max)
                                rs = gt.tile([1, 1], F32)
                                nc.vector.reciprocal(rs[:], ssum[:])
                                wg_sb = gt.tile([1, 8], F32)
                                nc.vector.memset(wg_sb[:], 0.0)
                                for aa in range(2):
                                    nc.vector.tensor_tensor(out=wg_sb[0:1, aa:aa + 1], in0=lab[0:1, aa:aa + 1],
                                                            in1=rs[:], op=AOT.mult)
                                nc.sync.dma_start(wg_local[:], wg_sb[:])

                        emitted = 0
                        pin_done = False
                        for (s, j, nblk) in call_plan:
                            xg_tiles[s][j] = issue_call(gp, "xg", ix1_sb[a, s], 
                                s_tab[a][s * TH: TH + s * (Nq - TH), :], NH2, NH2,
                                cfg.L1B, s, j, nblk, bufs=4)
                            emitted += 1
                            if emitted == 2 and not pin_done:
                                l1_window(0, WORDER[0])
                                pin_done = True
                            if emitted == 4 and a == 1:
                                nc.gpsimd.collective_compute("AllGather", AOT.bypass, replica_groups=RG,
                                                             ins=[wg_local.opt()], outs=[wg_all.opt()])
                                wg7 = gt.tile([1, 8], F32, name="wg7")
                                nc.sync.dma_start(wg7[:], wg_all[NC - 1:NC, :])
                                psb = gps.tile([P, 2], F32, name="psb")
                                nc.tensor.matmul(psb[:], lhsT=ones_row[:], rhs=wg7[0:1, 0:2],
                                                 start=True, stop=True)
                                nc.vector.tensor_copy(wgbc[:], psb[:])
                        for wi, w in enumerate(WORDER):
                            if wi == 0:
                                continue
                            l1_window(wi, w)

                # ---- q table write + all-gather
                nc.sync.dma_start(q_local[:].rearrange("(w p) c -> p w c", p=P)[:, :, 0:2 * NCLASS], q_sb[:])
                nc.gpsimd.collective_compute("AllGather", AOT.bypass, replica_groups=RG,
                                             ins=[q_local.opt()], outs=[q_tab.opt()])

            # ---- phase 3: L2 spmm -> Q tiles, then outputs
            with tc.tile_pool(name="l2keep", bufs=1) as l2p:
                Qsb = [l2p.tile([P, NT * 2 * NCLASS], F32, name=f"Qsb{a}") for a in range(2)]
                p_sb = [l2p.tile([P, NT * NCLASS], F32, name=f"p_sb{a}") for a in range(2)]
                out_sb = l2p.tile([P, NT * NCLASS], F32, name="out_sb")
                with tc.tile_pool(name="l2g", bufs=1) as g2p, \
                     tc.tile_pool(name="l2sc", bufs=1) as s2p, \
                     tc.tile_pool(name="l2ps", bufs=2, space="PSUM") as l2ps, \
                     tc.tile_pool(name="fin", bufs=4) as fp, \
                     tc.tile_pool(name="fin_ps", bufs=1, space="PSUM") as fps:
                    for a in range(2):
                        xq_tiles = {s: {} for s in streams}
                        call_plan = plan_calls(cfg.L2B)

                        def l2_window(wi, w, a=a, xq_tiles=xq_tiles):
                            ps = l2ps.tile([P, 2 * NCLASS], F32, tag="l2ps", bufs=3)
                            nblk_tot = sum(cfg.BPW[s] for s in streams)
                            ki = 0
                            for s in streams:
                                bpw = cfg.BPW[s]
                                Sc = s2p.tile([P, bpw * P], BF16, tag=f"sc2{s}", name=f"sc2{s}", bufs=4)
                                eng = nc.scalar if s == 0 else nc.sync
                                eng.dma_start(Sc[:], t_dest[a, s].ap()[:, wi * bpw * P:(wi + 1) * bpw * P])
                                for k in range(bpw):
                                    b = wi * bpw + k
                                    xq = xq_tiles[s][b // cfg.L2B]
                                    sl = b % cfg.L2B
                                    nc.tensor.matmul(ps[:], lhsT=Sc[:, k * P:(k + 1) * P],
                                                     rhs=xq[:, sl, 0:2 * NCLASS],
                                                     start=(ki == 0), stop=(ki == nblk_tot - 1))
                                    ki += 1
                            nc.vector.tensor_copy(Qsb[a][:, w * 2 * NCLASS:(w + 1) * 2 * NCLASS], ps[:])
                            if a == 1 and w == NT - 1:
                                # gate-2 SB computation (collective issued in the call stream)
                                for aa in range(2):
                                    wq_ = (NT - 1) * 2 * NCLASS
                                    q11 = Qsb[aa][:, wq_:wq_ + NCLASS]
                                    q12 = Qsb[aa][:, wq_ + NCLASS:wq_ + 2 * NCLASS]
                                    tmp1 = fp.tile([P, NCLASS], F32, tag="tmp1", name="tmp1")
                                    tmp2 = fp.tile([P, NCLASS], F32, tag="tmp2", name="tmp2")
                                    nc.vector.tensor_scalar(out=tmp1[:], in0=q11, scalar1=wgbc[:, 0:1],
                                                            scalar2=None, op0=AOT.mult)
                                    nc.vector.tensor_scalar(out=tmp2[:], in0=q12, scalar1=wgbc[:, 1:2],
                                                            scalar2=None, op0=AOT.mult)
                                    nc.vector.tensor_tensor(out=tmp1[:], in0=tmp1[:], in1=tmp2[:], op=AOT.add)
                                    nc.vector.tensor_tensor(
                                        out=p_sb[aa][:, (NT - 1) * NCLASS:NT * NCLASS],
                                        in0=tmp1[:], in1=b2bc[:], op=AOT.add)
                                lastw = (NT - 1) * NCLASS
                                lab2 = fp.tile([1, 2], F32, tag="lab2", bufs=1)
                                junk = fp.tile([1, NCLASS], F32, tag="junk", bufs=1)
                                dots = fp.tile([1, 2], F32, tag="dots", bufs=1)
                                p_last = [fp.tile([1, NCLASS], F32, tag=f"p_last{aa}", bufs=1, name=f"p_last{aa}")
                                          for aa in range(2)]
                                for aa in range(2):
                                    nc.sync.dma_start(p_last[aa][:],
                                                      p_sb[aa][PIN_SLOT:PIN_SLOT + 1, lastw:lastw + NCLASS])
                                for aa, gw in ((0, g1w), (1, g2w)):
                                    nc.vector.tensor_tensor(out=junk[:], in0=p_last[aa][:], in1=gw[:], op=AOT.mult)
                                    nc.vector.tensor_reduce(out=dots[0:1, aa:aa + 1], in_=junk[:],
                                                            op=AOT.add, axis=mybir.AxisListType.X)
                                    nc.scalar.activation(lab2[0:1, aa:aa + 1], dots[0:1, aa:aa + 1],
                                                         ACT.Sigmoid, bias=gb[0:1, 2 + aa:3 + aa], scale=1.0)
                                ssum2 = fp.tile([1, 1], F32, tag="ssum2", bufs=1)
                                nc.vector.tensor_tensor(out=ssum2[:], in0=lab2[0:1, 0:1], in1=lab2[0:1, 1:2], op=AOT.add)
                                rs2 = fp.tile([1, 1], F32, tag="rs2", bufs=1)
                                ag_sb = fp.tile([1, 8], F32, tag="ag_sb", bufs=1)
                                nc.vector.tensor_scalar(out=ssum2[:], in0=ssum2[:], scalar1=EPS, scalar2=None, op0=AOT.max)
                                nc.vector.reciprocal(rs2[:], ssum2[:])
                                nc.vector.memset(ag_sb[:], 0.0)
                                for aa in range(2):
                                    nc.vector.tensor_tensor(out=ag_sb[0:1, aa:aa + 1], in0=lab2[0:1, aa:aa + 1],
                                                            in1=rs2[:], op=AOT.mult)
                                nc.sync.dma_start(ag_local[:], ag_sb[:])
                            if w != NT - 1:
                                # interleaved finish: p_sb[a][w] (and out_sb[w] once both views done)
                                q11 = Qsb[a][:, w * 2 * NCLASS:w * 2 * NCLASS + NCLASS]
                                q12 = Qsb[a][:, w * 2 * NCLASS + NCLASS:(w + 1) * 2 * NCLASS]
                                tmp1 = fp.tile([P, NCLASS], F32, tag="tmp1", name="tmp1")
                                tmp2 = fp.tile([P, NCLASS], F32, tag="tmp2", name="tmp2")
                                nc.vector.tensor_scalar(out=tmp1[:], in0=q11, scalar1=wgbc[:, 0:1],
                                                        scalar2=None, op0=AOT.mult)
                                nc.vector.tensor_scalar(out=tmp2[:], in0=q12, scalar1=wgbc[:, 1:2],
                                                        scalar2=None, op0=AOT.mult)
                                nc.vector.tensor_tensor(out=tmp1[:], in0=tmp1[:], in1=tmp2[:], op=AOT.add)
                                nc.vector.tensor_tensor(out=p_sb[a][:, w * NCLASS:(w + 1) * NCLASS],
                                                        in0=tmp1[:], in1=b2bc[:], op=AOT.add)
                                if a == 1:
                                    ws = bass.ds(w * NCLASS, NCLASS)
                                    tmp3 = fp.tile([P, NCLASS], F32, tag="tmp1", name="tmp1")
                                    tmp4 = fp.tile([P, NCLASS], F32, tag="tmp2", name="tmp2")
                                    nc.vector.tensor_scalar(out=tmp3[:], in0=p_sb[0][:, ws], scalar1=agbc[:, 0:1],
                                                            scalar2=None, op0=AOT.mult)
                                    nc.vector.tensor_scalar(out=tmp4[:], in0=p_sb[1][:, ws], scalar1=agbc[:, 1:2],
                                                            scalar2=None, op0=AOT.mult)
                                    nc.vector.tensor_tensor(out=out_sb[:, ws], in0=tmp3[:], in1=tmp4[:], op=AOT.add)

                        emitted = 0
                        pin_done = False
                        for (s, j, nblk) in call_plan:
                            xq_tiles[s][j] = issue_call(g2p, "xq", ix2_sb[a, s],
                                q_tab[s * TH: TH + s * (Nq - TH), :], QW, QW,
                                cfg.L2B, s, j, nblk, bufs=8)
                            emitted += 1
                            if emitted == 2 and not pin_done:
                                l2_window(0, WORDER[0])
                                pin_done = True
                            if emitted == 4 and a == 1:
                                nc.gpsimd.collective_compute("AllGather", AOT.bypass, replica_groups=RG,
                                                             ins=[ag_local.opt()], outs=[ag_all.opt()])
                                ag7 = fp.tile([1, 8], F32, tag="ag7", bufs=1)
                                nc.sync.dma_start(ag7[:], ag_all[NC - 1:NC, :])
                                psb2 = fps.tile([P, 2], F32, name="psb2")
                                nc.tensor.matmul(psb2[:], lhsT=ones_row[:], rhs=ag7[0:1, 0:2],
                                                 start=True, stop=True)
                                nc.vector.tensor_copy(agbc[:], psb2[:])
                        for wi, w in enumerate(WORDER):
                            if wi == 0:
                                continue
                            l2_window(wi, w)

                    # ---- phase 4: pin-window output row (rest interleaved above)
                    w = NT - 1
                    ws = bass.ds(w * NCLASS, NCLASS)
                    tmp1 = fp.tile([P, NCLASS], F32, tag="tmp1", name="tmp1")
                    tmp2 = fp.tile([P, NCLASS], F32, tag="tmp2", name="tmp2")
                    nc.vector.tensor_scalar(out=tmp1[:], in0=p_sb[0][:, ws], scalar1=agbc[:, 0:1],
                                            scalar2=None, op0=AOT.mult)
                    nc.vector.tensor_scalar(out=tmp2[:], in0=p_sb[1][:, ws], scalar1=agbc[:, 1:2],
                                            scalar2=None, op0=AOT.mult)
                    nc.vector.tensor_tensor(out=out_sb[:, ws], in0=tmp1[:], in1=tmp2[:], op=AOT.add)

                    nc.sync.dma_start(t_out["o_g1"].ap(), p_sb[0][:])
                    nc.sync.dma_start(t_out["o_g2"].ap(), p_sb[1][:])
                    nc.sync.dma_start(t_out["o_out"].ap(), out_sb[:])

    nc.compile()
    return nc


# ---------------------------------------------------------------- driver

def postprocess(results, perms, cfg):
    N, Nc, NT, NCLASS = cfg.N, cfg.Nc, cfg.NT, cfg.NCLASS
    outs = {}
    for name in ("o_out", "o_g1", "o_g2"):
        full = np.empty((N, NCLASS), np.float32)
        for cc in range(cfg.NCORES):
            arr = results[cc][name].reshape(P, NT, NCLASS).transpose(1, 0, 2).reshape(cfg.Ncp, NCLASS)
            full[cc * Nc:(cc + 1) * Nc] = arr[perms[cc]]
        outs[name] = full
    return outs["o_out"], outs["o_g1"], outs["o_g2"]


def run(inputs, cfg=DEFAULT_CFG, nc=None, trace=False):
    in_maps, perms = prepare(inputs, cfg)
    if nc is None:
        use_bias = any(np.any(np.asarray(inputs[k])) for k in ("b1a", "b1b"))
        nc = build(cfg, use_bias)
    res = bass_utils.run_bass_kernel_spmd(
        nc, in_maps, core_ids=list(range(cfg.NCORES)), trace=trace)
    return postprocess(res.results, perms, cfg), res


_CACHE = {}


def kernel(**inputs):
    cfg = DEFAULT_CFG
    if "nc" not in _CACHE:
        use_bias = any(np.any(np.asarray(inputs[k])) for k in ("b1a", "b1b"))
        _CACHE["nc"] = build(cfg, use_bias)
    (out, g1, g2), _ = run(inputs, cfg=cfg, nc=_CACHE["nc"])
    return out, g1, g2


# revision 27
# speedup vs baseline: 1.0476x; 1.0476x over previous
"""Self-contained CLAGCN kernel for 8 Trainium2 NeuronCores (v2).

kernel(**inputs) takes the full (unsharded) numpy inputs and returns the
full outputs (out, gnn_prop1, gnn_prop2), matching reference().

Node rows are partitioned contiguously across the 8 cores (graph
parallel); edges live on the core that owns their destination row.
First-layer supports (x @ W1, both branches) are computed per-shard as
bf16 and all-gathered per view; each core runs its spmm as dma_gather
(source-row fetch) + one-hot matmul segment-sum into PSUM windows of 128
destinations.  One-hot "S" blocks are generated on-device by the vector
engine from per-slot (dest, val) pairs (iota compare), instead of being
streamed from HBM.  The second layer repeats the pattern on the gathered
[h1W2 | h2W2] table.  Tiny sigmoid gates are computed on the core owning
the last node and broadcast with small all-gathers; scalar broadcasts go
through a 1xK matmul so the gpsimd engine stays dedicated to gather
descriptor generation (the critical path).
"""

import sys

sys.path.insert(0, "/opt/trn_rl_repo")
import numpy as np
import ml_dtypes

import concourse.bass as bass
import concourse.mybir as mybir
import concourse.tile as tile
from concourse import bacc
from concourse import bass_utils

BF16 = mybir.dt.bfloat16
F32 = mybir.dt.float32
I16 = mybir.dt.int16
AOT = mybir.AluOpType
ACT = mybir.ActivationFunctionType
P = 128
EPS = 1e-12
PIN_SLOT = 96  # partition slot of the last local node


class Cfg:
    def __init__(self, N=50000, NFEAT=256, NHID=128, NCLASS=16, NCORES=8,
                 TH_CORES=5, BPW_LO=11, BPW_HI=7, L1B=18, L2B=16):
        self.N, self.NFEAT, self.NHID, self.NCLASS, self.NCORES = N, NFEAT, NHID, NCLASS, NCORES
        assert N % NCORES == 0
        self.Nc = N // NCORES                 # nodes per core
        self.NT = (self.Nc + P - 1) // P      # dest windows per core
        self.Ncp = self.NT * P                # padded local nodes
        self.TH_CORES = TH_CORES              # lo/hi table split at this core boundary
        self.TH = TH_CORES * self.Ncp         # row threshold in the gathered tables
        self.BPW = (BPW_LO, BPW_HI)           # edge blocks per window per stream
        self.NBS = (self.NT * BPW_LO, self.NT * BPW_HI)
        self.L1B = L1B                        # L1 gather batch (blocks per dma_gather)
        self.L2B = L2B
        self.NH2 = 2 * NHID                   # 256: concat branch a/b (per-view table row)
        self.QW = 128                         # q table row elems (bf16; 2*NCLASS used)
        self.Nq = NCORES * self.Ncp
        assert self.TH < 32768 and self.Nq - self.TH < 32768


DEFAULT_CFG = Cfg()


# ---------------------------------------------------------------- host prep

def _pack_core(r1, r2, cfg):
    """Assign local dest nodes to (window, slot), balancing joint degree.
    Returns perm[Nc] -> permuted index (window*128 + slot)."""
    import heapq
    Nc, NT = cfg.Nc, cfg.NT
    deg1 = np.bincount(r1, minlength=Nc)
    deg2 = np.bincount(r2, minlength=Nc)
    degj = deg1 + deg2
    last = Nc - 1
    order = np.argsort(-degj, kind="stable")
    order = order[order != last]

    win_members = [[] for _ in range(NT)]
    win_members[NT - 1].append(last)
    heap = [(int(degj[last]) if w == NT - 1 else 0, w) for w in range(NT)]
    heapq.heapify(heap)
    for d in order:
        while True:
            load, w = heapq.heappop(heap)
            if len(win_members[w]) < P:
                break
        win_members[w].append(int(d))
        heapq.heappush(heap, (load + int(degj[d]), w))

    perm = np.empty(Nc, np.int64)
    for w in range(NT):
        slots = [s for s in range(P) if not (w == NT - 1 and s == PIN_SLOT)]
        members = win_members[w]
        if w == NT - 1:
            perm[last] = w * P + PIN_SLOT
            members = [d for d in members if d != last]
        assert len(members) <= len(slots)
        for s, d in zip(slots, members):
            perm[d] = w * P + s
    return perm


def _stream_arrays(r, c, v, perm, bpw, cfg, pad_col=0):
    """Lay out one stream's edges into slot arrays (window blocks in WORDER
    processing order so gather calls are consumed sequentially).
    Returns (col_global [NBs*128], destval [128, NBs*2] f32)."""
    NT = cfg.NT
    NBs = NT * bpw
    worder = [NT - 1] + list(range(NT - 1))
    worder_pos = np.empty(NT, np.int64)
    for i, w in enumerate(worder):
        worder_pos[w] = i
    pidx = perm[r]
    w_e = worder_pos[pidx // P]
    order = np.argsort(w_e, kind="stable")
    w_s = w_e[order]
    counts = np.bincount(w_s, minlength=NT)
    assert counts.max() <= bpw * P, f"stream window overflow: {counts.max()} > {bpw * P}"
    starts = np.concatenate([[0], np.cumsum(counts)[:-1]])
    within = np.arange(len(order)) - starts[w_s]
    slot = w_s * (bpw * P) + within

    col_flat = np.full(NBs * P, pad_col, np.int64)
    dest_flat = np.zeros(NBs * P, np.float32)
    val_flat = np.zeros(NBs * P, np.float32)
    col_flat[slot] = c[order]
    dest_flat[slot] = (pidx[order] % P).astype(np.float32)
    val_flat[slot] = v[order]
    return (col_flat,
            dest_flat.reshape(NBs, P).T.copy(),
            val_flat.reshape(NBs, P).T.copy())


def _wrap16(rows):
    """int16 dma_gather index layout: [128, n/16], position i -> [i%16, i//16]."""
    assert rows.min() >= 0 and rows.max() < 32768, (rows.min(), rows.max())
    return np.tile(rows.astype(np.int16).reshape(-1, 16).T, (8, 1)).copy()


def prepare(inputs, cfg):
    """Split + pack everything.  Returns (in_maps, perms)."""
    bf = ml_dtypes.bfloat16
    N, Nc, Ncp, NFEAT = cfg.N, cfg.Nc, cfg.Ncp, cfg.NFEAT
    NC = cfg.NCORES

    row1 = np.asarray(inputs["row1"]); col1 = np.asarray(inputs["col1"]); ew1 = np.asarray(inputs["ew1"])
    row2 = np.asarray(inputs["row2"]); col2 = np.asarray(inputs["col2"]); ew2 = np.asarray(inputs["ew2"])

    E1 = [np.nonzero(row1 // Nc == cc)[0] for cc in range(NC)]
    E2 = [np.nonzero(row2 // Nc == cc)[0] for cc in range(NC)]

    perms = []
    for cc in range(NC):
        r1 = row1[E1[cc]] - cc * Nc
        r2 = row2[E2[cc]] - cc * Nc
        perms.append(_pack_core(r1, r2, cfg))

    W1a = np.asarray(inputs["W1a"]).astype(bf)
    W1b = np.asarray(inputs["W1b"]).astype(bf)
    Wq = np.asarray(inputs["W2"]).astype(bf)
    bias_row = np.concatenate([np.asarray(inputs["b1a"]), np.asarray(inputs["b1b"])])[None, :].astype(np.float32)
    b2bc = np.broadcast_to(np.asarray(inputs["b2"])[None, :], (P, cfg.NCLASS)).astype(np.float32).copy()
    fw1 = np.asarray(inputs["fw1_w"])[:, 0][None, :].astype(np.float32).copy()
    fw2 = np.asarray(inputs["fw2_w"])[:, 0][None, :].astype(np.float32).copy()
    g1w = np.asarray(inputs["g1_w"]).T.astype(np.float32).copy()
    g2w = np.asarray(inputs["g2_w"]).T.astype(np.float32).copy()
    gb = np.array([[float(inputs["fw1_b"][0]), float(inputs["fw2_b"][0]),
                    float(inputs["g1_b"][0]), float(inputs["g2_b"][0])]], np.float32)
    ones_row = np.ones((1, P), np.float32)
    ident = np.eye(P, dtype=np.float32).astype(bf)

    def xT(name, cc):
        xs = np.asarray(inputs[name])[cc * Nc:(cc + 1) * Nc].astype(bf)
        out = np.zeros((NFEAT, Ncp), bf)
        out[:, :Nc] = xs.T
        return out

    in_maps = []
    for cc in range(NC):
        perm = perms[cc]
        im = {
            "xT1a": xT("x1a", cc), "xT1b": xT("x1b", cc),
            "xT2a": xT("x2a", cc), "xT2b": xT("x2b", cc),
            "W1a": W1a, "W1b": W1b, "Wq": Wq,
            "bias_row": bias_row, "b2bc": b2bc,
            "fw1": fw1, "fw2": fw2, "g1w": g1w, "g2w": g2w, "gb": gb,
            "ones_row": ones_row, "ident": ident,
        }
        for a, (Ea, rowa, cola, ewa) in enumerate(((E1[cc], row1, col1, ew1),
                                                   (E2[cc], row2, col2, ew2))):
            r = rowa[Ea] - cc * Nc
            c = cola[Ea]
            v = ewa[Ea]
            lo = (c // Nc) < cfg.TH_CORES
            for s, mask in enumerate((lo, ~lo)):
                colg, dest, val = _stream_arrays(r[mask], c[mask], v[mask],
                                                 perm, cfg.BPW[s], cfg,
                                                 pad_col=s * cfg.TH_CORES * Nc)
                nbs = cfg.NBS[s]
                S = np.zeros((P, nbs * P), ml_dtypes.bfloat16)
                pidx_r = np.arange(P)[:, None]
                colidx = np.arange(nbs)[None, :] * P + dest.astype(np.int64)
                S[pidx_r, colidx] = val.astype(ml_dtypes.bfloat16)
                im[f"S{a}{s}"] = S
                oc = colg // Nc
                loc = colg % Nc
                srow = oc * Ncp + loc
                qrow = np.empty_like(colg)
                for o in range(NC):
                    m = oc == o
                    qrow[m] = o * Ncp + perms[o][loc[m]]
                off = s * cfg.TH
                im[f"ix1_{a}{s}"] = _wrap16(srow - off)
                im[f"ix2_{a}{s}"] = _wrap16(qrow - off)
        in_maps.append(im)
    return in_maps, perms


# ---------------------------------------------------------------- program

def build(cfg, use_bias=False):
    NFEAT, NHID, NCLASS = cfg.NFEAT, cfg.NHID, cfg.NCLASS
    NT, Ncp, TH, Nq = cfg.NT, cfg.Ncp, cfg.TH, cfg.Nq
    NH2, QW = cfg.NH2, cfg.QW
    NC = cfg.NCORES
    RG = [list(range(NC))]
    KH = NFEAT // P
    assert KH == 2 and NHID == P
    WORDER = [NT - 1] + list(range(NT - 1))   # pin window first (gates early)
    NQUEUE = 4

    nc = bacc.Bacc("TRN2", target_bir_lowering=False, debug=False, num_devices=NC,
                   num_swdge_queues=4)

    t_x = {n: nc.dram_tensor(n, [NFEAT, Ncp], BF16, kind="ExternalInput")
           for n in ("xT1a", "xT1b", "xT2a", "xT2b")}
    t_W1a = nc.dram_tensor("W1a", [NFEAT, NHID], BF16, kind="ExternalInput")
    t_W1b = nc.dram_tensor("W1b", [NFEAT, NHID], BF16, kind="ExternalInput")
    t_Wq = nc.dram_tensor("Wq", [NH2, NCLASS], BF16, kind="ExternalInput")
    t_bias_row = nc.dram_tensor("bias_row", [1, NH2], F32, kind="ExternalInput")
    t_b2bc = nc.dram_tensor("b2bc", [P, NCLASS], F32, kind="ExternalInput")
    t_fw1 = nc.dram_tensor("fw1", [1, NH2], F32, kind="ExternalInput")
    t_fw2 = nc.dram_tensor("fw2", [1, NH2], F32, kind="ExternalInput")
    t_g1w = nc.dram_tensor("g1w", [1, NCLASS], F32, kind="ExternalInput")
    t_g2w = nc.dram_tensor("g2w", [1, NCLASS], F32, kind="ExternalInput")
    t_gb = nc.dram_tensor("gb", [1, 4], F32, kind="ExternalInput")
    t_ones = nc.dram_tensor("ones_row", [1, P], F32, kind="ExternalInput")
    t_ident = nc.dram_tensor("ident", [P, P], BF16, kind="ExternalInput")
    t_dest, t_ix1, t_ix2 = {}, {}, {}
    for a in range(2):
        for s in range(2):
            nbs = cfg.NBS[s]
            t_dest[a, s] = nc.dram_tensor(f"S{a}{s}", [P, nbs * P], BF16, kind="ExternalInput")
            t_ix1[a, s] = nc.dram_tensor(f"ix1_{a}{s}", [P, nbs * 8], I16, kind="ExternalInput")
            t_ix2[a, s] = nc.dram_tensor(f"ix2_{a}{s}", [P, nbs * 8], I16, kind="ExternalInput")
    t_out = {n: nc.dram_tensor(n, [P, NT * NCLASS], F32, kind="ExternalOutput")
             for n in ("o_out", "o_g1", "o_g2")}

    streams = (0, 1)
    qcount = [0]

    def plan_calls(B):
        """Merged (stream, call_idx, nblk) order advancing window coverage evenly."""
        ncalls = {s: (cfg.NBS[s] + B - 1) // B for s in streams}
        nxt = {s: 0 for s in streams}
        order = []
        while any(nxt[s] < ncalls[s] for s in streams):
            cand = [s for s in streams if nxt[s] < ncalls[s]]
            s = min(cand, key=lambda s: nxt[s] * B / cfg.BPW[s])
            j = nxt[s]
            order.append((s, j, min(B, cfg.NBS[s] - j * B)))
            nxt[s] += 1
        return order

    def issue_call(gp, tag, ix_sb, table_ap, elem, step, B, s, j, nblk, bufs):
        xg = gp.tile([P, B, elem], BF16, tag=f"{tag}{s}", name=f"{tag}{s}", bufs=bufs)
        nc.gpsimd.dma_gather(
            out_ap=xg[:, 0:nblk, :],
            in_ap=table_ap,
            idxs_ap=ix_sb[:, j * B * 8: (j * B + nblk) * 8],
            num_idxs=nblk * P, num_idxs_reg=nblk * P,
            elem_size=elem, elem_step=step, single_packet=False,
            queue_num=qcount[0] % NQUEUE)
        qcount[0] += 1
        return xg

    with tile.TileContext(nc) as tc:
        with tc.tile_pool(name="dram", bufs=1, space="DRAM") as dram, \
             tc.tile_pool(name="const", bufs=1) as cp:

            s_local = [dram.tile([Ncp, NH2], BF16, name=f"s_local{v}") for v in range(2)]
            s_tab = [dram.tile([Nq, NH2], BF16, addr_space="Shared", name=f"s_tab{v}") for v in range(2)]
            q_local = dram.tile([Ncp, QW], BF16)
            q_tab = dram.tile([Nq, QW], BF16, addr_space="Shared")
            wg_local = dram.tile([1, 8], F32)
            wg_all = dram.tile([NC, 8], F32, addr_space="Shared")
            ag_local = dram.tile([1, 8], F32)
            ag_all = dram.tile([NC, 8], F32, addr_space="Shared")

            w1a_sb = [cp.tile([P, NHID], BF16, name=f"w1a{kh}") for kh in range(KH)]
            w1b_sb = [cp.tile([P, NHID], BF16, name=f"w1b{kh}") for kh in range(KH)]
            wq_sb = [cp.tile([P, NCLASS], BF16, name=f"wq{kh}") for kh in range(KH)]
            for kh in range(KH):
                nc.sync.dma_start(w1a_sb[kh][:], t_W1a.ap()[kh * P:(kh + 1) * P, :])
                nc.sync.dma_start(w1b_sb[kh][:], t_W1b.ap()[kh * P:(kh + 1) * P, :])
                nc.sync.dma_start(wq_sb[kh][:], t_Wq.ap()[kh * P:(kh + 1) * P, :])
            bias_row = cp.tile([1, NH2], F32); nc.sync.dma_start(bias_row[:], t_bias_row.ap())
            b2bc = cp.tile([P, NCLASS], F32); nc.sync.dma_start(b2bc[:], t_b2bc.ap())
            fw1 = cp.tile([1, NH2], F32); nc.sync.dma_start(fw1[:], t_fw1.ap())
            fw2 = cp.tile([1, NH2], F32); nc.sync.dma_start(fw2[:], t_fw2.ap())
            g1w = cp.tile([1, NCLASS], F32); nc.sync.dma_start(g1w[:], t_g1w.ap())
            g2w = cp.tile([1, NCLASS], F32); nc.sync.dma_start(g2w[:], t_g2w.ap())
            gb = cp.tile([1, 4], F32); nc.sync.dma_start(gb[:], t_gb.ap())
            ones_row = cp.tile([1, P], F32); nc.sync.dma_start(ones_row[:], t_ones.ap())
            ident = cp.tile([P, P], BF16); nc.sync.dma_start(ident[:], t_ident.ap())
            wgbc = cp.tile([P, 2], F32)
            agbc = cp.tile([P, 2], F32)

            ix1_sb, ix2_sb = {}, {}
            for a in range(2):
                for s in streams:
                    nbs = cfg.NBS[s]
                    ix1_sb[a, s] = cp.tile([P, nbs * 8], I16, name=f"ix1sb{a}{s}")
                    ix2_sb[a, s] = cp.tile([P, nbs * 8], I16, name=f"ix2sb{a}{s}")

            # q accumulator (both views); table cols 2*NCLASS..QW are never read
            q_sb = cp.tile([P, NT, 2 * NCLASS], BF16)

            # ---- phase 1: dense supports s_v = [x_va @ W1a | x_vb @ W1b]
            with tc.tile_pool(name="dense", bufs=1) as dp, \
                 tc.tile_pool(name="dense_ps", bufs=2, space="PSUM") as dps:
                for v in range(2):
                    xa = [dp.tile([P, Ncp], BF16, tag=f"xa{kh}", name=f"xa{kh}") for kh in range(KH)]
                    xb = [dp.tile([P, Ncp], BF16, tag=f"xb{kh}", name=f"xb{kh}") for kh in range(KH)]
                    for kh in range(KH):
                        nc.sync.dma_start(xa[kh][:], t_x[f"xT{v+1}a"].ap()[kh * P:(kh + 1) * P, :])
                        nc.sync.dma_start(xb[kh][:], t_x[f"xT{v+1}b"].ap()[kh * P:(kh + 1) * P, :])
                    for t in range(NT):
                        ts = bass.ts(t, P)
                        ps = dps.tile([P, NH2], F32, tag="dps")
                        for kh in range(KH):
                            nc.tensor.matmul(ps[:, 0:NHID], lhsT=xa[kh][:, ts], rhs=w1a_sb[kh][:],
                                             start=(kh == 0), stop=(kh == KH - 1))
                        for kh in range(KH):
                            nc.tensor.matmul(ps[:, NHID:NH2], lhsT=xb[kh][:, ts], rhs=w1b_sb[kh][:],
                                             start=(kh == 0), stop=(kh == KH - 1))
                        s_sb = dp.tile([P, NH2], BF16, tag="s_sb", bufs=3)
                        nc.vector.tensor_copy(s_sb[:], ps[:])
                        nc.sync.dma_start(s_local[v][t * P:(t + 1) * P, :], s_sb[:])
                    # per-view all-gather as soon as this view's supports are done
                    nc.gpsimd.collective_compute(
                        "AllGather", AOT.bypass, replica_groups=RG,
                        ins=[s_local[v].opt()], outs=[s_tab[v].opt()])
                    if v == 0:
                        for a in range(2):
                            for s in streams:
                                nc.sync.dma_start(ix1_sb[a, s][:], t_ix1[a, s].ap())
                        for a in range(2):
                            for s in streams:
                                nc.sync.dma_start(ix2_sb[a, s][:], t_ix2[a, s].ap())

            # ---- phase 2: L1 spmm -> hT (transposed, bf16, SBUF-resident), fused q
            with tc.tile_pool(name="hpool", bufs=1) as hp:
                h_nm = [hp.tile([P, NT, NH2], BF16, name=f"hnm{a}") for a in range(2)]

                with tc.tile_pool(name="l1g", bufs=1) as gp, \
                     tc.tile_pool(name="l1sc", bufs=1) as scp, \
                     tc.tile_pool(name="l1ps", bufs=2, space="PSUM") as l1ps, \
                     tc.tile_pool(name="qps_pool", bufs=2, space="PSUM") as qps, \
                     tc.tile_pool(name="gate", bufs=1) as gt, \
                     tc.tile_pool(name="gate_ps", bufs=1, space="PSUM") as gps:
                    for a in range(2):
                        xg_tiles = {s: {} for s in streams}
                        call_plan = plan_calls(cfg.L1B)
                        q_pending = []

                        def emit_q(w, a=a):
                            trs = []
                            for kh in range(KH):
                                pst = qps.tile([P, P], F32, tag="pst")
                                nc.tensor.matmul(pst[:], lhsT=h_nm[a][:, w, kh * P:(kh + 1) * P],
                                                 rhs=ident[:], start=True, stop=True)
                                tr = scp.tile([P, P], BF16, tag="tr", bufs=3, name="tr")
                                nc.vector.tensor_copy(tr[:], pst[:])
                                trs.append(tr)
                            qp = qps.tile([P, NCLASS], F32, tag="qps")
                            for kh in range(KH):
                                nc.tensor.matmul(qp[:], lhsT=trs[kh][:], rhs=wq_sb[kh][:],
                                                 start=(kh == 0), stop=(kh == KH - 1))
                            nc.vector.tensor_copy(q_sb[:, w, a * NCLASS:(a + 1) * NCLASS], qp[:])

                        def l1_window(wi, w, a=a, xg_tiles=xg_tiles):
                            ps = l1ps.tile([P, NH2], F32, tag="l1ps", bufs=2)
                            if use_bias:
                                nc.tensor.matmul(ps[:], lhsT=ones_row[:], rhs=bias_row[:],
                                                 start=True, stop=False)
                            nblk_tot = sum(cfg.BPW[s] for s in streams)
                            ki = 0
                            for s in streams:
                                bpw = cfg.BPW[s]
                                Sc = scp.tile([P, bpw * P], BF16, tag=f"sc{s}", name=f"sc{s}", bufs=4)
                                eng = nc.scalar if s == 0 else nc.sync
                                eng.dma_start(Sc[:], t_dest[a, s].ap()[:, wi * bpw * P:(wi + 1) * bpw * P])
                                for k in range(bpw):
                                    b = wi * bpw + k
                                    xg = xg_tiles[s][b // cfg.L1B]
                                    sl = b % cfg.L1B
                                    nc.tensor.matmul(ps[:], lhsT=Sc[:, k * P:(k + 1) * P],
                                                     rhs=xg[:, sl, :],
                                                     start=(not use_bias and ki == 0),
                                                     stop=(ki == nblk_tot - 1))
                                    ki += 1
                            nc.vector.tensor_scalar(out=h_nm[a][:, w, :], in0=ps[:],
                                                    scalar1=0.0, scalar2=None, op0=AOT.max)
                            # q pipeline: emit the PREVIOUS window's transposes + q
                            # matmuls now, so the tensor engine never waits on
                            # this window's DVE eviction.
                            if q_pending:
                                emit_q(q_pending.pop())
                            q_pending.append(w)
                            if a == 1 and w == NT - 1:
                                # gate-1 SB computation (collective issued in the call stream)
                                lab = gt.tile([1, 2], F32)
                                junkg = gt.tile([1, NH2], F32, name="junkg")
                                dotg = gt.tile([1, 2], F32, name="dotg")
                                h_last = [gt.tile([1, NH2], BF16, name=f"h_last{aa}") for aa in range(2)]
                                for aa in range(2):
                                    nc.sync.dma_start(h_last[aa][:], h_nm[aa][PIN_SLOT:PIN_SLOT + 1, NT - 1, :])
                                for aa, fw in ((0, fw1), (1, fw2)):
                                    nc.vector.tensor_tensor(out=junkg[:], in0=h_last[aa][:],
                                                            in1=fw[:], op=AOT.mult)
                                    nc.vector.tensor_reduce(out=dotg[0:1, aa:aa + 1], in_=junkg[:],
                                                            op=AOT.add, axis=mybir.AxisListType.X)
                                    nc.scalar.activation(lab[0:1, aa:aa + 1], dotg[0:1, aa:aa + 1],
                                                         ACT.Sigmoid, bias=gb[0:1, aa:aa + 1], scale=1.0)
                                ssum = gt.tile([1, 1], F32)
                                nc.vector.tensor_tensor(out=ssum[:], in0=lab[0:1, 0:1], in1=lab[0:1, 1:2], op=AOT.add)
                                nc.vector.tensor_scalar(out=ssum[:], in0=ssum[:], scalar1=EPS, scalar2=None, op0=AOT.max)
                                rs = gt.tile([1, 1], F32)
                                nc.vector.reciprocal(rs[:], ssum[:])
                                wg_sb = gt.tile([1, 8], F32)
                                nc.vector.memset(wg_sb[:], 0.0)
                                for aa in range(2):
                                    nc.vector.tensor_tensor(out=wg_sb[0:1, aa:aa + 1], in0=lab[0:1, aa:aa + 1],
                                                            in1=rs[:], op=AOT.mult)
                                nc.sync.dma_start(wg_local[:], wg_sb[:])

                        emitted = 0
                        pin_done = False
                        for (s, j, nblk) in call_plan:
                            xg_tiles[s][j] = issue_call(gp, "xg", ix1_sb[a, s], 
                                s_tab[a][s * TH: TH + s * (Nq - TH), :], NH2, NH2,
                                cfg.L1B, s, j, nblk, bufs=4)
                            emitted += 1
                            if emitted == 2 and not pin_done:
                                l1_window(0, WORDER[0])
                                pin_done = True
                            if emitted == 4 and a == 1:
                                nc.gpsimd.collective_compute("AllGather", AOT.bypass, replica_groups=RG,
                                                             ins=[wg_local.opt()], outs=[wg_all.opt()])
                                wg7 = gt.tile([1, 8], F32, name="wg7")
                                nc.sync.dma_start(wg7[:], wg_all[NC - 1:NC, :])
                                psb = gps.tile([P, 2], F32, name="psb")
                                nc.tensor.matmul(psb[:], lhsT=ones_row[:], rhs=wg7[0:1, 0:2],
                                                 start=True, stop=True)
                                nc.vector.tensor_copy(wgbc[:], psb[:])
                        for wi, w in enumerate(WORDER):
                            if wi == 0:
                                continue
                            l1_window(wi, w)
                        if q_pending:
                            emit_q(q_pending.pop())

                # ---- q table write + all-gather
                nc.sync.dma_start(q_local[:].rearrange("(w p) c -> p w c", p=P)[:, :, 0:2 * NCLASS], q_sb[:])
                nc.gpsimd.collective_compute("AllGather", AOT.bypass, replica_groups=RG,
                                             ins=[q_local.opt()], outs=[q_tab.opt()])

            # ---- phase 3: L2 spmm -> Q tiles, then outputs
            with tc.tile_pool(name="l2keep", bufs=1) as l2p:
                Qsb = [l2p.tile([P, NT * 2 * NCLASS], F32, name=f"Qsb{a}") for a in range(2)]
                p_sb = [l2p.tile([P, NT * NCLASS], F32, name=f"p_sb{a}") for a in range(2)]
                out_sb = l2p.tile([P, NT * NCLASS], F32, name="out_sb")
                with tc.tile_pool(name="l2g", bufs=1) as g2p, \
                     tc.tile_pool(name="l2sc", bufs=1) as s2p, \
                     tc.tile_pool(name="l2ps", bufs=2, space="PSUM") as l2ps, \
                     tc.tile_pool(name="fin", bufs=4) as fp, \
                     tc.tile_pool(name="fin_ps", bufs=1, space="PSUM") as fps:
                    for a in range(2):
                        xq_tiles = {s: {} for s in streams}
                        call_plan = plan_calls(cfg.L2B)

                        def l2_window(wi, w, a=a, xq_tiles=xq_tiles):
                            ps = l2ps.tile([P, 2 * NCLASS], F32, tag="l2ps", bufs=3)
                            nblk_tot = sum(cfg.BPW[s] for s in streams)
                            ki = 0
                            for s in streams:
                                bpw = cfg.BPW[s]
                                Sc = s2p.tile([P, bpw * P], BF16, tag=f"sc2{s}", name=f"sc2{s}", bufs=4)
                                eng = nc.scalar if s == 0 else nc.sync
                                eng.dma_start(Sc[:], t_dest[a, s].ap()[:, wi * bpw * P:(wi + 1) * bpw * P])
                                for k in range(bpw):
                                    b = wi * bpw + k
                                    xq = xq_tiles[s][b // cfg.L2B]
                                    sl = b % cfg.L2B
                                    nc.tensor.matmul(ps[:], lhsT=Sc[:, k * P:(k + 1) * P],
                                                     rhs=xq[:, sl, 0:2 * NCLASS],
                                                     start=(ki == 0), stop=(ki == nblk_tot - 1))
                                    ki += 1
                            nc.vector.tensor_copy(Qsb[a][:, w * 2 * NCLASS:(w + 1) * 2 * NCLASS], ps[:])
                            if a == 1 and w == NT - 1:
                                # gate-2 SB computation (collective issued in the call stream)
                                for aa in range(2):
                                    wq_ = (NT - 1) * 2 * NCLASS
                                    q11 = Qsb[aa][:, wq_:wq_ + NCLASS]
                                    q12 = Qsb[aa][:, wq_ + NCLASS:wq_ + 2 * NCLASS]
                                    tmp1 = fp.tile([P, NCLASS], F32, tag="tmp1", name="tmp1")
                                    tmp2 = fp.tile([P, NCLASS], F32, tag="tmp2", name="tmp2")
                                    nc.vector.tensor_scalar(out=tmp1[:], in0=q11, scalar1=wgbc[:, 0:1],
                                                            scalar2=None, op0=AOT.mult)
                                    nc.vector.tensor_scalar(out=tmp2[:], in0=q12, scalar1=wgbc[:, 1:2],
                                                            scalar2=None, op0=AOT.mult)
                                    nc.vector.tensor_tensor(out=tmp1[:], in0=tmp1[:], in1=tmp2[:], op=AOT.add)
                                    nc.vector.tensor_tensor(
                                        out=p_sb[aa][:, (NT - 1) * NCLASS:NT * NCLASS],
                                        in0=tmp1[:], in1=b2bc[:], op=AOT.add)
                                lastw = (NT - 1) * NCLASS
                                lab2 = fp.tile([1, 2], F32, tag="lab2", bufs=1)
                                junk = fp.tile([1, NCLASS], F32, tag="junk", bufs=1)
                                dots = fp.tile([1, 2], F32, tag="dots", bufs=1)
                                p_last = [fp.tile([1, NCLASS], F32, tag=f"p_last{aa}", bufs=1, name=f"p_last{aa}")
                                          for aa in range(2)]
                                for aa in range(2):
                                    nc.sync.dma_start(p_last[aa][:],
                                                      p_sb[aa][PIN_SLOT:PIN_SLOT + 1, lastw:lastw + NCLASS])
                                for aa, gw in ((0, g1w), (1, g2w)):
                                    nc.vector.tensor_tensor(out=junk[:], in0=p_last[aa][:], in1=gw[:], op=AOT.mult)
                                    nc.vector.tensor_reduce(out=dots[0:1, aa:aa + 1], in_=junk[:],
                                                            op=AOT.add, axis=mybir.AxisListType.X)
                                    nc.scalar.activation(lab2[0:1, aa:aa + 1], dots[0:1, aa:aa + 1],
                                                         ACT.Sigmoid, bias=gb[0:1, 2 + aa:3 + aa], scale=1.0)
                                ssum2 = fp.tile([1, 1], F32, tag="ssum2", bufs=1)
                                nc.vector.tensor_tensor(out=ssum2[:], in0=lab2[0:1, 0:1], in1=lab2[0:1, 1:2], op=AOT.add)
                                rs2 = fp.tile([1, 1], F32, tag="rs2", bufs=1)
                                ag_sb = fp.tile([1, 8], F32, tag="ag_sb", bufs=1)
                                nc.vector.tensor_scalar(out=ssum2[:], in0=ssum2[:], scalar1=EPS, scalar2=None, op0=AOT.max)
                                nc.vector.reciprocal(rs2[:], ssum2[:])
                                nc.vector.memset(ag_sb[:], 0.0)
                                for aa in range(2):
                                    nc.vector.tensor_tensor(out=ag_sb[0:1, aa:aa + 1], in0=lab2[0:1, aa:aa + 1],
                                                            in1=rs2[:], op=AOT.mult)
                                nc.sync.dma_start(ag_local[:], ag_sb[:])
                            if w != NT - 1:
                                # interleaved finish: p_sb[a][w] (and out_sb[w] once both views done)
                                q11 = Qsb[a][:, w * 2 * NCLASS:w * 2 * NCLASS + NCLASS]
                                q12 = Qsb[a][:, w * 2 * NCLASS + NCLASS:(w + 1) * 2 * NCLASS]
                                tmp1 = fp.tile([P, NCLASS], F32, tag="tmp1", name="tmp1")
                                tmp2 = fp.tile([P, NCLASS], F32, tag="tmp2", name="tmp2")
                                nc.vector.tensor_scalar(out=tmp1[:], in0=q11, scalar1=wgbc[:, 0:1],
                                                        scalar2=None, op0=AOT.mult)
                                nc.vector.tensor_scalar(out=tmp2[:], in0=q12, scalar1=wgbc[:, 1:2],
                                                        scalar2=None, op0=AOT.mult)
                                nc.vector.tensor_tensor(out=tmp1[:], in0=tmp1[:], in1=tmp2[:], op=AOT.add)
                                nc.vector.tensor_tensor(out=p_sb[a][:, w * NCLASS:(w + 1) * NCLASS],
                                                        in0=tmp1[:], in1=b2bc[:], op=AOT.add)
                                if a == 1:
                                    ws = bass.ds(w * NCLASS, NCLASS)
                                    tmp3 = fp.tile([P, NCLASS], F32, tag="tmp1", name="tmp1")
                                    tmp4 = fp.tile([P, NCLASS], F32, tag="tmp2", name="tmp2")
                                    nc.vector.tensor_scalar(out=tmp3[:], in0=p_sb[0][:, ws], scalar1=agbc[:, 0:1],
                                                            scalar2=None, op0=AOT.mult)
                                    nc.vector.tensor_scalar(out=tmp4[:], in0=p_sb[1][:, ws], scalar1=agbc[:, 1:2],
                                                            scalar2=None, op0=AOT.mult)
                                    nc.vector.tensor_tensor(out=out_sb[:, ws], in0=tmp3[:], in1=tmp4[:], op=AOT.add)

                        emitted = 0
                        pin_done = False
                        for (s, j, nblk) in call_plan:
                            xq_tiles[s][j] = issue_call(g2p, "xq", ix2_sb[a, s],
                                q_tab[s * TH: TH + s * (Nq - TH), :], QW, QW,
                                cfg.L2B, s, j, nblk, bufs=8)
                            emitted += 1
                            if emitted == 2 and not pin_done:
                                l2_window(0, WORDER[0])
                                pin_done = True
                            if emitted == 4 and a == 1:
                                nc.gpsimd.collective_compute("AllGather", AOT.bypass, replica_groups=RG,
                                                             ins=[ag_local.opt()], outs=[ag_all.opt()])
                                ag7 = fp.tile([1, 8], F32, tag="ag7", bufs=1)
                                nc.sync.dma_start(ag7[:], ag_all[NC - 1:NC, :])
                                psb2 = fps.tile([P, 2], F32, name="psb2")
                                nc.tensor.matmul(psb2[:], lhsT=ones_row[:], rhs=ag7[0:1, 0:2],
                                                 start=True, stop=True)
                                nc.vector.tensor_copy(agbc[:], psb2[:])
                        for wi, w in enumerate(WORDER):
                            if wi == 0:
                                continue
                            l2_window(wi, w)

                    # ---- phase 4: pin-window output row (rest interleaved above)
                    w = NT - 1
                    ws = bass.ds(w * NCLASS, NCLASS)
                    tmp1 = fp.tile([P, NCLASS], F32, tag="tmp1", name="tmp1")
                    tmp2 = fp.tile([P, NCLASS], F32, tag="tmp2", name="tmp2")
                    nc.vector.tensor_scalar(out=tmp1[:], in0=p_sb[0][:, ws], scalar1=agbc[:, 0:1],
                                            scalar2=None, op0=AOT.mult)
                    nc.vector.tensor_scalar(out=tmp2[:], in0=p_sb[1][:, ws], scalar1=agbc[:, 1:2],
                                            scalar2=None, op0=AOT.mult)
                    nc.vector.tensor_tensor(out=out_sb[:, ws], in0=tmp1[:], in1=tmp2[:], op=AOT.add)

                    nc.sync.dma_start(t_out["o_g1"].ap(), p_sb[0][:])
                    nc.sync.dma_start(t_out["o_g2"].ap(), p_sb[1][:])
                    nc.sync.dma_start(t_out["o_out"].ap(), out_sb[:])

    nc.compile()
    return nc


# ---------------------------------------------------------------- driver

def postprocess(results, perms, cfg):
    N, Nc, NT, NCLASS = cfg.N, cfg.Nc, cfg.NT, cfg.NCLASS
    outs = {}
    for name in ("o_out", "o_g1", "o_g2"):
        full = np.empty((N, NCLASS), np.float32)
        for cc in range(cfg.NCORES):
            arr = results[cc][name].reshape(P, NT, NCLASS).transpose(1, 0, 2).reshape(cfg.Ncp, NCLASS)
            full[cc * Nc:(cc + 1) * Nc] = arr[perms[cc]]
        outs[name] = full
    return outs["o_out"], outs["o_g1"], outs["o_g2"]


def run(inputs, cfg=DEFAULT_CFG, nc=None, trace=False):
    in_maps, perms = prepare(inputs, cfg)
    if nc is None:
        use_bias = any(np.any(np.asarray(inputs[k])) for k in ("b1a", "b1b"))
        nc = build(cfg, use_bias)
    res = bass_utils.run_bass_kernel_spmd(
        nc, in_maps, core_ids=list(range(cfg.NCORES)), trace=trace)
    return postprocess(res.results, perms, cfg), res


_CACHE = {}


def kernel(**inputs):
    cfg = DEFAULT_CFG
    if "nc" not in _CACHE:
        use_bias = any(np.any(np.asarray(inputs[k])) for k in ("b1a", "b1b"))
        _CACHE["nc"] = build(cfg, use_bias)
    (out, g1, g2), _ = run(inputs, cfg=cfg, nc=_CACHE["nc"])
    return out, g1, g2
